# revision 13
# baseline (speedup 1.0000x reference)
"""HD95 loss kernel for Trainium2 (Bass/Tile), 8 NeuronCores.

Reference semantics: per image, threshold pred/true at 0.5, compact nonzero
pixel indices in row-major order, split each point list into blocks of 1000,
and for every (point, opposite-side block) pair take the min Euclidean
distance; the HD95 is the 95th linear-interpolation quantile over all finite
such mins (both directions), averaged over the batch.

Device algorithm (per image & direction): grid-EDT. Every query is a pixel
of the 96x96 grid, so the device computes, for ALL grid pixels (y, x) and
every ref block, min_c [ (y - t_c)^2 + g[x, blk, c] ] where t_c = b0+c runs
over the block's <=23 candidate image rows and g[x, blk, c] is the 1-D
row-EDT min_a (x - a)^2 over the block's points in row t_c (host-prepared,
O(rows x 96) two-pointer work). Since y^2 is constant inside the min, the
device evaluates v = -2ty + (t^2 + g) per candidate with one contraction-3
bf16 matmul column [-2t, Bhi, Blo] against stationary y-features [y, 1, 1]
(B = t^2 + g split into a multiple-of-128 part plus a <128 remainder, so
every product is bf16-exact and the fp32 PSUM sum is the exact integer v).
DVE min-reduces over candidates give v[y, (x, blk)]; the host gathers the
actual query pixels, adds y^2 back, takes sqrt, and computes the final
quantile.

Core mapping: 8 cores = 4 (image x direction) jobs x 2 x-halves of the
grid. Per core: 12 matmuls (460 cols), 3 min-reduces, 3 input + 3 output
DMAs. Host does O(N) compaction, the tiny row-EDT table, and the O(50k)
quantile; device does all O(grid x window) distance evaluation and minima.
"""

import numpy as np

H = 96
W = 96
BLK = 1000        # reference cdist block size
NBLK = 5          # blocks per side (asserted from the data regime)
CAND = 23         # candidate image rows per block window (max actual span)
XH = 48           # x columns per core (half the grid)
CXM = 4           # x values per matmul chunk
CPB = NBLK * CAND           # 115 candidate cols per x value
MMF = CXM * CPB             # 460 matmul free size (<= 512 PSUM bank)
NMM = XH // CXM             # 12 matmuls per core
BPT = 4                     # matmul chunks (PSUM banks) per tile
NT = NMM // BPT             # 3 PSUM tiles -> 3 reduces, 3 output DMAs
NCOL = XH * CPB             # 5520 rhs cols per core
BIG = float(2 ** 26)  # sentinel (bf16-exact, >> max real d^2 of ~20k)
NCORES = 8

_CACHE = {}


def _build_nc():
    import concourse.bacc as bacc
    import concourse.mybir as mybir
    import concourse.tile as tile

    f32 = mybir.dt.float32
    bf16 = mybir.dt.bfloat16
    # Bacc (not raw Bass): its compile() runs move_matmul_waits_to_ldweights
    # + generate_event_semaphores, which legalize multi-wait instructions
    # (TRN2 allows at most one sync wait per instruction).
    nc = bacc.Bacc("TRN2", target_bir_lowering=False, debug=False)

    pack = nc.declare_dram_parameter("pack", [3, 96 + NCOL], bf16, isOutput=False)
    mins = nc.declare_dram_parameter("mins", [96, NT * BPT * CXM * NBLK], f32,
                                     isOutput=True)

    X = mybir.AxisListType.X
    MIN = mybir.AluOpType.min

    with tile.TileContext(nc) as tc:
        with (
            tc.tile_pool(name="const", bufs=1) as const,
            tc.tile_pool(name="ps", bufs=2, space="PSUM") as psp,
        ):
            t_in = const.tile([3, 96 + NCOL], bf16)
            t_out = const.tile([96, NT * BPT * CXM * NBLK], f32)
            t_lhsT = t_in[:, 0:96]

            # input DMA split into staggered pieces on DIFFERENT engine
            # queues: issues run in parallel and the first matmul only
            # waits for the small first piece (lhsT + 1 matmul chunk)
            bounds = [0, 96 + MMF, 96 + 4 * MMF, 96 + 8 * MMF, 96 + NCOL]
            for q in range(4):
                sl = slice(bounds[q], bounds[q + 1])
                nc.sync.dma_start(t_in[:, sl], pack[:, sl])

            for t in range(NT):
                ps = psp.tile([96, BPT, 512], f32, tag="ps")
                for k in range(BPT):
                    c0 = 96 + (t * BPT + k) * MMF
                    nc.tensor.matmul(
                        ps[:, k, 0:MMF],
                        t_lhsT,
                        t_in[:, c0 : c0 + MMF],
                        start=True,
                        stop=True,
                    )
                # [96, BPT, (CXM, NBLK, CAND)] -> min over candidates; the
                # last tile reduces in two halves so the final (critical-
                # path) output DMA covers only a quarter of the columns
                splits = (
                    [(0, BPT)] if t < NT - 1
                    else [(0, BPT // 2), (BPT // 2, BPT)]
                )
                for b0, b1 in splits:
                    red_in = ps[:, b0:b1, 0:MMF].rearrange(
                        "p b (x j c) -> p b (x j) c", x=CXM, j=NBLK, c=CAND
                    )
                    o0 = (t * BPT + b0) * CXM * NBLK
                    o1 = (t * BPT + b1) * CXM * NBLK
                    nc.vector.tensor_reduce(
                        t_out[:, o0:o1], red_in, axis=X, op=MIN
                    )
                    nc.sync.dma_start(mins[:, o0:o1], t_out[:, o0:o1])

    nc.compile()
    return nc


def _get_nc():
    if "nc" not in _CACHE:
        _CACHE["nc"] = _build_nc()
    return _CACHE["nc"]


def _bf16(a):
    from ml_dtypes import bfloat16

    return np.asarray(a, np.float32).astype(bfloat16)


def _hilo(v):
    """Split integer-valued array into (multiple-of-128, remainder<128)."""
    v = np.asarray(v, np.float64)
    lo = np.mod(v, 128.0)
    return (v - lo).astype(np.float32), lo.astype(np.float32)


def _side_points(img):
    """Compacted nonzero pixel coords, row-major ascending (matches
    jnp.nonzero order)."""
    m = (np.asarray(img) > 0.5).reshape(-1)
    idx = np.nonzero(m)[0]
    ys = (idx // W).astype(np.int64)
    xs = (idx % W).astype(np.int64)
    return ys, xs


def _g_table(r_ys, r_xs):
    """Host row-EDT: B[x, blk, c] = t^2 + min_a (x-a)^2 over block blk's
    points in image row t = b0(blk)+c, or the BIG sentinel for empty
    candidate rows. Returns None if the data falls outside the compiled
    regime (not 5 blocks, or a block row-span > CAND)."""
    cnt = len(r_ys)
    if not (4 * BLK < cnt <= NBLK * BLK):
        return None
    B = np.full((96, NBLK, CAND), BIG, np.float64)
    xg = np.arange(96)
    for j in range(NBLK):
        lo, hi = j * BLK, min((j + 1) * BLK, cnt)
        ys_b, xs_b = r_ys[lo:hi], r_xs[lo:hi]
        b0 = int(ys_b[0])
        if int(ys_b[-1]) - b0 + 1 > CAND:
            return None
        # per-candidate-row slices of the (row-major sorted) point list
        starts = np.searchsorted(ys_b, b0 + np.arange(CAND), side="left")
        ends = np.searchsorted(ys_b, b0 + np.arange(CAND), side="right")
        for c in range(CAND):
            s, e = starts[c], ends[c]
            if s == e:
                continue  # empty candidate row -> sentinel
            a = xs_b[s:e]  # ascending x's present in this row-block
            i = np.searchsorted(a, xg).clip(1, e - s - 1) if e - s > 1 else \
                np.zeros(96, np.int64)
            if e - s > 1:
                d = np.minimum(np.abs(xg - a[i - 1]), np.abs(a[i] - xg))
            else:
                d = np.abs(xg - a[0])
            t = float(b0 + c)
            B[:, j, c] = t * t + d.astype(np.float64) ** 2
    return B


def _build_job_packs(r_ys, r_xs):
    """Packed [5, 96+NCOL] bf16 inputs for the two cores of one job."""
    Bt = _g_table(r_ys, r_xs)
    if Bt is None:
        return None
    y = np.arange(96, dtype=np.float64)
    one = np.ones(96, np.float32)
    lhsT = np.stack([y.astype(np.float32), one, one])  # [3, 96]

    b0s = np.array([int(r_ys[j * BLK]) for j in range(NBLK)], np.float64)
    t = b0s[:, None] + np.arange(CAND)[None, :]        # [NBLK, CAND]
    tneg2 = np.broadcast_to(-2.0 * t, (96, NBLK, CAND))
    Bhi, Blo = _hilo(Bt)
    # rhs rows [-2t, Bhi, Blo] per column (x, blk, c)
    rhs = np.stack([tneg2.astype(np.float32), Bhi, Blo])
    rhs = rhs.reshape(3, 96, CPB)

    packs = []
    for h in range(2):
        cols = rhs[:, h * XH : (h + 1) * XH, :].reshape(3, NCOL)
        packs.append({"pack": _bf16(np.concatenate([lhsT, cols], axis=1))})
    return packs


def _quantile95(vals):
    """torch.quantile / jnp.nanquantile 'linear' on finite values."""
    v = np.sort(np.asarray(vals, np.float64))
    n = v.size
    pos = 0.95 * (n - 1)
    lo = int(np.floor(pos))
    hi = min(lo + 1, n - 1)
    frac = pos - lo
    return v[lo] * (1.0 - frac) + v[hi] * frac


def _hd95_numpy_fallback(pred, true):
    """Pure-numpy path for data outside the compiled regime."""
    p_ys, p_xs = _side_points(pred)
    t_ys, t_xs = _side_points(true)
    if len(p_ys) == 0 or len(t_ys) == 0:
        return None
    pc = np.stack([p_ys, p_xs], -1).astype(np.float32)
    tc = np.stack([t_ys, t_xs], -1).astype(np.float32)
    vals = []
    for qc, rc in ((pc, tc), (tc, pc)):
        nbr = (len(rc) + BLK - 1) // BLK
        for jb in range(nbr):
            b = rc[jb * BLK : (jb + 1) * BLK]
            d2 = (
                (qc * qc).sum(-1)[:, None]
                + (b * b).sum(-1)[None, :]
                - 2.0 * (qc @ b.T)
            )
            vals.append(np.sqrt(np.maximum(d2.min(1), 0.0).astype(np.float32)))
    return _quantile95(np.concatenate(vals))


def _run_device(in_maps, trace=False):
    from concourse.bass_utils import run_bass_kernel_spmd

    nc = _get_nc()
    return run_bass_kernel_spmd(nc, in_maps, list(range(NCORES)), trace=trace)


def kernel(input, target, _trace=False, _results_out=None):
    input = np.asarray(input)
    target = np.asarray(target)
    nimg = input.shape[0]

    # jobs: (image, direction). dir 0: queries=pred, refs=true (row mins);
    # dir 1: queries=true, refs=pred (col mins). 2 cores per job (x halves).
    jobs = []
    in_maps = []
    fallback = {}
    ok_mask = []
    pts = {}
    for i in range(nimg):
        pts[i, 0] = _side_points(input[i])
        pts[i, 1] = _side_points(target[i])
        ok = len(pts[i, 0][0]) > 0 and len(pts[i, 1][0]) > 0
        ok_mask.append(ok)
        if not ok:
            continue
        built_row = _build_job_packs(*pts[i, 1])  # refs = true
        built_col = _build_job_packs(*pts[i, 0])  # refs = pred
        if built_row is None or built_col is None or nimg != 2:
            fallback[i] = _hd95_numpy_fallback(input[i], target[i])
            continue
        jobs.append((i, 0))
        in_maps.extend(built_row)
        jobs.append((i, 1))
        in_maps.extend(built_col)

    hds = {}
    if jobs:
        while len(in_maps) < NCORES:  # pad to the full 8-core SPMD launch
            in_maps.append({k: v.copy() for k, v in in_maps[0].items()})
        res = _run_device(in_maps[:NCORES], trace=_trace)
        if _results_out is not None:
            _results_out.append(res)
        per_img_vals = {}
        for j, (img, dr) in enumerate(jobs):
            o0 = res.results[2 * j]["mins"]      # [96, XH*NBLK] x in [0,48)
            o1 = res.results[2 * j + 1]["mins"]  # x in [48,96)
            d2 = np.concatenate(
                [o0.reshape(96, XH, NBLK), o1.reshape(96, XH, NBLK)], axis=1
            )  # [y, x, blk]
            q_ys, q_xs = pts[img, dr]
            # add back the y^2 term dropped from the device min
            qv = d2[q_ys, q_xs, :] + (q_ys * q_ys)[:, None].astype(np.float32)
            assert qv.max() < 2.0 ** 25, "sentinel leaked into mins"
            per_img_vals.setdefault(img, []).append(
                np.sqrt(qv.astype(np.float32)).ravel()
            )
        for img, chunks in per_img_vals.items():
            hds[img] = _quantile95(np.concatenate(chunks))
    hds.update(fallback)

    n_ok = sum(ok_mask)
    if n_ok == 0:
        return np.float32(np.inf)
    total = sum(hds[i] for i in range(nimg) if ok_mask[i])
    return np.float32(total / n_ok)
